# revision 4
# baseline (speedup 1.0000x reference)
"""GraphSAGE (3-layer, mean-aggr, L2-normalize, ReLU) on 8 Trainium2 NeuronCores.

Sharding: nodes split across 8 cores (1250 each, padded to 1280). Per layer:
  y = x @ Wl.T computed on own rows -> AllGather y_full [10240, d] ->
  per 128-dst-node block: dma_gather of (deduped, sorted) source rows of
  y_full + indicator-matrix matmuls on the PE (segment mean with 1/deg folded
  into the indicator), plus x @ Wr.T and a rank-1 bias matmul into the same
  PSUM accumulator -> ACT/DVE epilogue (L2 normalize + ReLU) ->
  PE-transpose back into the feature-major x for the next layer.

kernel(**inputs) takes the full unsharded inputs and returns the full
[10000, 256] float32 output.
"""

import math

import numpy as np

import concourse.bass as bass
import concourse.mybir as mybir
import concourse.tile as tile
from concourse import bacc
from concourse import bass_utils
from concourse.masks import make_identity

P = 128
N = 10000
E = 160000
DIMS = [512, 512, 512, 256]
NCORES = 8
NP = N // NCORES  # 1250 real nodes per core
NB = 10  # dst blocks of 128 per core
NPP = NB * P  # 1280 padded nodes per core
CT = 6  # gather chunk size in k-tiles (128 edges each)

F32 = mybir.dt.float32
I16 = mybir.dt.int16


def _preprocess(edge_index):
    """Build per-core gather indices + indicator matrices from edge_index.

    Returns (K_blk[NB], gidx[NCORES, 128, SW] int16, ind[NCORES, 128, SK] f32).
    """
    src = np.asarray(edge_index[0], dtype=np.int64)
    dst = np.asarray(edge_index[1], dtype=np.int64)

    deg = np.bincount(dst, minlength=N)
    dinv = np.zeros(N, dtype=np.float64)
    nz = deg > 0
    dinv[nz] = 1.0 / deg[nz]

    ps = (src // NP) * NPP + (src % NP)  # padded global source row in y_full
    core = dst // NP
    ld = dst - core * NP
    blk = ld // P
    slot = ld - blk * P
    g = core * NB + blk  # global block id 0..79

    # dedup (block, source) pairs -> gather rows; sorted by (block, ps)
    key = g * (NCORES * NPP) + ps
    uniq, row_of_edge = np.unique(key, return_inverse=True)
    g_of_row = uniq // (NCORES * NPP)
    ps_of_row = uniq % (NCORES * NPP)
    nrows = np.bincount(g_of_row, minlength=NCORES * NB)
    row_start = np.zeros(NCORES * NB, dtype=np.int64)
    row_start[1:] = np.cumsum(nrows)[:-1]
    row_local = np.arange(len(uniq)) - row_start[g_of_row]

    # uniform-per-block padded row count (max over cores, multiple of 128)
    per_block = nrows.reshape(NCORES, NB).max(axis=0)
    K_blk = ((per_block + P - 1) // P * P).astype(np.int64)
    K_blk = np.maximum(K_blk, P)
    off = np.zeros(NB, dtype=np.int64)
    off[1:] = np.cumsum(K_blk)[:-1]
    SK = int(K_blk.sum())
    SW = SK // 16

    gidx = np.zeros((NCORES, 16, SW), dtype=np.int16)
    r_core = g_of_row // NB
    r_blk = g_of_row % NB
    gidx[r_core, row_local % 16, off[r_blk] // 16 + row_local // 16] = \
        ps_of_row.astype(np.int16)
    gidx = np.tile(gidx, (1, 8, 1))  # replicate 16 -> 128 partitions

    ind = np.zeros((NCORES, P, SK), dtype=np.float32)
    e_row = row_local[row_of_edge]  # row within block for each edge
    e_col = off[blk] + (e_row // P) * P + slot
    np.add.at(ind, (core, e_row % P, e_col), dinv[dst].astype(np.float32))

    return K_blk, gidx, ind


def _build(K_blk):
    """Build + compile the SPMD bass program (identical across 8 cores)."""
    SK = int(K_blk.sum())
    SW = SK // 16
    off = np.zeros(NB, dtype=np.int64)
    off[1:] = np.cumsum(K_blk)[:-1]

    nc = bacc.Bacc("TRN2", target_bir_lowering=False, debug=False,
                   num_devices=NCORES)

    xT0 = nc.dram_tensor("xT0", [DIMS[0], NPP], F32, kind="ExternalInput").ap()
    gidx_d = nc.dram_tensor("gidx", [P, SW], I16, kind="ExternalInput").ap()
    ind_d = nc.dram_tensor("ind", [P, SK], F32, kind="ExternalInput").ap()
    wl_d, wr_d, b_d = [], [], []
    for l in range(3):
        din, dout = DIMS[l], DIMS[l + 1]
        wl_d.append(nc.dram_tensor(f"wlT{l}", [din, dout], F32,
                                   kind="ExternalInput").ap())
        wr_d.append(nc.dram_tensor(f"wrT{l}", [din, dout], F32,
                                   kind="ExternalInput").ap())
        b_d.append(nc.dram_tensor(f"b{l}", [1, dout], F32,
                                  kind="ExternalInput").ap())
    out_d = nc.dram_tensor("out", [NPP, DIMS[3]], F32, kind="ExternalOutput").ap()

    ystage, yfull = [], []
    for l in range(3):
        dout = DIMS[l + 1]
        ystage.append(nc.dram_tensor(f"ystage{l}", [NPP, dout], F32,
                                     kind="Internal").ap())
        yfull.append(nc.dram_tensor(f"yfull{l}", [NCORES * NPP, dout], F32,
                                    kind="Internal", addr_space="Shared").ap())

    with tile.TileContext(nc) as tc:
        with (
            tc.tile_pool(name="const", bufs=1) as cpool,
            tc.tile_pool(name="xpool", bufs=1) as xpool,
            tc.tile_pool(name="wpool", bufs=2) as wpool,
            tc.tile_pool(name="ypool", bufs=3) as ypool,
            tc.tile_pool(name="gpool", bufs=3) as gpool,
            tc.tile_pool(name="ipool", bufs=3) as ipool,
            tc.tile_pool(name="epool", bufs=2) as epool,
            tc.tile_pool(name="psy", bufs=2, space="PSUM") as psy,
            tc.tile_pool(name="pso", bufs=2, space="PSUM") as pso,
            tc.tile_pool(name="pst", bufs=2, space="PSUM") as pst,
        ):
            ident = cpool.tile([P, P], F32)
            make_identity(nc, ident[:])
            ones = cpool.tile([1, P], F32)
            nc.vector.memset(ones[:], 1.0)
            gidx_sb = cpool.tile([P, SW], I16)
            nc.sync.dma_start(out=gidx_sb[:], in_=gidx_d[:])

            xA = xpool.tile([P, 4, NPP], F32, tag="xA")
            xB = xpool.tile([P, 4, NPP], F32, tag="xB")
            nc.sync.dma_start(
                out=xA[:], in_=xT0.rearrange("(t p) m -> p t m", p=P))

            for l in range(3):
                din, dout = DIMS[l], DIMS[l + 1]
                kt = din // P
                xs = [xA, xB, xA][l]
                xd = [xB, xA, None][l]

                wl_sb = wpool.tile([P, kt, dout], F32, tag="wl")
                wr_sb = wpool.tile([P, kt, dout], F32, tag="wr")
                bias_sb = wpool.tile([1, dout], F32, tag="bias")
                nc.sync.dma_start(
                    out=wl_sb[:], in_=wl_d[l].rearrange("(t p) d -> p t d", p=P))
                nc.sync.dma_start(
                    out=wr_sb[:], in_=wr_d[l].rearrange("(t p) d -> p t d", p=P))
                nc.sync.dma_start(out=bias_sb[:], in_=b_d[l][:])

                # ---- Phase A: y = x @ Wl.T for own rows, then AllGather ----
                for m in range(NB):
                    ps_y = psy.tile([P, dout], F32, tag="psy")
                    for k in range(kt):
                        nc.tensor.matmul(
                            ps_y[:],
                            lhsT=xs[:, k, m * P:(m + 1) * P],
                            rhs=wl_sb[:, k, :],
                            start=(k == 0), stop=(k == kt - 1))
                    y_sb = ypool.tile([P, dout], F32, tag="y")
                    nc.scalar.copy(y_sb[:], ps_y[:])
                    nc.sync.dma_start(
                        out=ystage[l][m * P:(m + 1) * P, :], in_=y_sb[:])

                nc.gpsimd.collective_compute(
                    "AllGather", mybir.AluOpType.bypass,
                    replica_groups=[list(range(NCORES))],
                    ins=[ystage[l][:]], outs=[yfull[l][:]])

                # ---- Phase B: per dst block: gather + segment-mean + Wr + b --
                for b in range(NB):
                    nt = int(K_blk[b]) // P
                    ps_o = pso.tile([P, dout], F32, tag="pso")
                    first = True
                    for c0 in range(0, nt, CT):
                        cs = min(CT, nt - c0)
                        g_sb = gpool.tile([P, CT, dout], F32, tag="g")
                        nc.gpsimd.dma_gather(
                            out_ap=g_sb[:, :cs, :],
                            in_ap=yfull[l][:],
                            idxs_ap=gidx_sb[:, (int(off[b]) + c0 * P) // 16:
                                            (int(off[b]) + (c0 + cs) * P) // 16],
                            num_idxs=cs * P,
                            num_idxs_reg=cs * P,
                            elem_size=dout)
                        i_sb = ipool.tile([P, CT * P], F32, tag="i")
                        nc.sync.dma_start(
                            out=i_sb[:, :cs * P],
                            in_=ind_d[:, int(off[b]) + c0 * P:
                                      int(off[b]) + (c0 + cs) * P])
                        for j in range(cs):
                            nc.tensor.matmul(
                                ps_o[:],
                                lhsT=i_sb[:, j * P:(j + 1) * P],
                                rhs=g_sb[:, j, :dout],
                                start=first, stop=False)
                            first = False
                    for k in range(kt):
                        nc.tensor.matmul(
                            ps_o[:],
                            lhsT=xs[:, k, b * P:(b + 1) * P],
                            rhs=wr_sb[:, k, :],
                            start=False, stop=False)
                    nc.tensor.matmul(
                        ps_o[:], lhsT=ones[:1, :], rhs=bias_sb[:1, :],
                        start=False, stop=True)

                    # epilogue: r -> relu(r) / max(||r||, eps)
                    sq = epool.tile([P, DIMS[1]], F32, tag="sq")
                    ssq = epool.tile([P, 1], F32, tag="ssq")
                    nc.scalar.activation(
                        out=sq[:, :dout], in_=ps_o[:],
                        func=mybir.ActivationFunctionType.Square,
                        accum_out=ssq[:])
                    nc.vector.tensor_scalar_max(ssq[:], ssq[:], 1e-24)
                    nrm = epool.tile([P, 1], F32, tag="nrm")
                    nc.scalar.sqrt(nrm[:], ssq[:])
                    rns = epool.tile([P, 1], F32, tag="rns")
                    nc.vector.reciprocal(rns[:], nrm[:])
                    xn = epool.tile([P, DIMS[1]], F32, tag="xn")
                    nc.scalar.activation(
                        out=xn[:, :dout], in_=ps_o[:],
                        func=mybir.ActivationFunctionType.Relu,
                        scale=rns[:])

                    if l < 2:
                        for k in range(dout // P):
                            ps_t = pst.tile([P, P], F32, tag="pst")
                            nc.tensor.transpose(
                                ps_t[:], xn[:, k * P:(k + 1) * P], ident[:])
                            nc.vector.tensor_copy(
                                out=xd[:, k, b * P:(b + 1) * P], in_=ps_t[:])
                    else:
                        nc.sync.dma_start(
                            out=out_d[b * P:(b + 1) * P, :], in_=xn[:, :dout])

    nc.compile()
    return nc


_CACHE = {}


def _prep(x, edge_index, Wl0, b0, Wr0, Wl1, b1, Wr1, Wl2, b2, Wr2):
    """Host preprocessing -> (compiled nc, per-core in_maps)."""
    x = np.asarray(x, dtype=np.float32)
    K_blk, gidx, ind = _preprocess(np.asarray(edge_index))

    key = tuple(K_blk.tolist())
    if key not in _CACHE:
        _CACHE[key] = _build(K_blk)
    nc = _CACHE[key]

    weights = {}
    for l, (Wl, bb, Wr) in enumerate([(Wl0, b0, Wr0), (Wl1, b1, Wr1),
                                      (Wl2, b2, Wr2)]):
        weights[f"wlT{l}"] = np.ascontiguousarray(
            np.asarray(Wl, dtype=np.float32).T)
        weights[f"wrT{l}"] = np.ascontiguousarray(
            np.asarray(Wr, dtype=np.float32).T)
        weights[f"b{l}"] = np.asarray(bb, dtype=np.float32).reshape(1, -1)

    in_maps = []
    for c in range(NCORES):
        xc = np.zeros((NPP, DIMS[0]), dtype=np.float32)
        xc[:NP] = x[c * NP:(c + 1) * NP]
        in_maps.append({
            "xT0": np.ascontiguousarray(xc.T),
            "gidx": np.ascontiguousarray(gidx[c]),
            "ind": np.ascontiguousarray(ind[c]),
            **weights,
        })
    return nc, in_maps


def kernel(x, edge_index, Wl0, b0, Wr0, Wl1, b1, Wr1, Wl2, b2, Wr2):
    nc, in_maps = _prep(x, edge_index, Wl0, b0, Wr0, Wl1, b1, Wr1,
                        Wl2, b2, Wr2)
    res = bass_utils.run_bass_kernel_spmd(
        nc, in_maps, core_ids=list(range(NCORES)))
    out = np.concatenate(
        [res.results[c]["out"][:NP] for c in range(NCORES)], axis=0)
    return out.astype(np.float32)


# revision 5
# speedup vs baseline: 1.1839x; 1.1839x over previous
"""GraphSAGE (3-layer, mean-aggr, L2-normalize, ReLU) on 8 Trainium2 NeuronCores.

Sharding: nodes split across 8 cores (1250 each, padded to 1280). Per layer:
  y = x @ Wl.T computed on own rows -> AllGather y_full [10240, d] ->
  per 128-dst-node block: dma_gather of (deduped, sorted) source rows of
  y_full + indicator-matrix matmuls on the PE (segment mean with 1/deg folded
  into the indicator), plus x @ Wr.T and a rank-1 bias matmul into the same
  PSUM accumulator -> ACT/DVE epilogue (L2 normalize + ReLU) ->
  PE-transpose back into the feature-major x for the next layer.

kernel(**inputs) takes the full unsharded inputs and returns the full
[10000, 256] float32 output.
"""

import math

import ml_dtypes
import numpy as np

import concourse.bass as bass
import concourse.mybir as mybir
import concourse.tile as tile
from concourse import bacc
from concourse import bass_utils
from concourse.masks import make_identity

P = 128
N = 10000
E = 160000
DIMS = [512, 512, 512, 256]
NCORES = 8
NP = N // NCORES  # 1250 real nodes per core
NB = 10  # dst blocks of 128 per core
NPP = NB * P  # 1280 padded nodes per core
CT = 6  # gather chunk size in k-tiles (128 edges each)

F32 = mybir.dt.float32
BF16 = mybir.dt.bfloat16
I16 = mybir.dt.int16


def _preprocess(edge_index):
    """Build per-core gather indices + indicator matrices from edge_index.

    Returns (K_blk[NB], gidx[NCORES, 128, SW] int16, ind[NCORES, 128, SK] f32).
    """
    src = np.asarray(edge_index[0], dtype=np.int64)
    dst = np.asarray(edge_index[1], dtype=np.int64)

    deg = np.bincount(dst, minlength=N)
    dinv = np.zeros(N, dtype=np.float64)
    nz = deg > 0
    dinv[nz] = 1.0 / deg[nz]

    ps = (src // NP) * NPP + (src % NP)  # padded global source row in y_full
    core = dst // NP
    ld = dst - core * NP
    blk = ld // P
    slot = ld - blk * P
    g = core * NB + blk  # global block id 0..79

    # dedup (block, source) pairs -> gather rows; sorted by (block, ps)
    key = g * (NCORES * NPP) + ps
    uniq, row_of_edge = np.unique(key, return_inverse=True)
    g_of_row = uniq // (NCORES * NPP)
    ps_of_row = uniq % (NCORES * NPP)
    nrows = np.bincount(g_of_row, minlength=NCORES * NB)
    row_start = np.zeros(NCORES * NB, dtype=np.int64)
    row_start[1:] = np.cumsum(nrows)[:-1]
    row_local = np.arange(len(uniq)) - row_start[g_of_row]

    # uniform-per-block padded row count (max over cores, multiple of 128)
    per_block = nrows.reshape(NCORES, NB).max(axis=0)
    K_blk = ((per_block + P - 1) // P * P).astype(np.int64)
    K_blk = np.maximum(K_blk, P)
    off = np.zeros(NB, dtype=np.int64)
    off[1:] = np.cumsum(K_blk)[:-1]
    SK = int(K_blk.sum())
    SW = SK // 16

    gidx = np.zeros((NCORES, 16, SW), dtype=np.int16)
    r_core = g_of_row // NB
    r_blk = g_of_row % NB
    gidx[r_core, row_local % 16, off[r_blk] // 16 + row_local // 16] = \
        ps_of_row.astype(np.int16)
    gidx = np.tile(gidx, (1, 8, 1))  # replicate 16 -> 128 partitions

    ind = np.zeros((NCORES, P, SK), dtype=np.float32)
    e_row = row_local[row_of_edge]  # row within block for each edge
    e_col = off[blk] + (e_row // P) * P + slot
    np.add.at(ind, (core, e_row % P, e_col), dinv[dst].astype(np.float32))

    return K_blk, gidx, ind.astype(ml_dtypes.bfloat16)


def _build(K_blk):
    """Build + compile the SPMD bass program (identical across 8 cores)."""
    SK = int(K_blk.sum())
    SW = SK // 16
    off = np.zeros(NB, dtype=np.int64)
    off[1:] = np.cumsum(K_blk)[:-1]

    nc = bacc.Bacc("TRN2", target_bir_lowering=False, debug=False,
                   num_devices=NCORES)

    xT0 = nc.dram_tensor("xT0", [DIMS[0], NPP], F32, kind="ExternalInput").ap()
    gidx_d = nc.dram_tensor("gidx", [P, SW], I16, kind="ExternalInput").ap()
    ind_d = nc.dram_tensor("ind", [P, SK], BF16, kind="ExternalInput").ap()
    wl_d, wr_d, b_d = [], [], []
    for l in range(3):
        din, dout = DIMS[l], DIMS[l + 1]
        wl_d.append(nc.dram_tensor(f"wlT{l}", [din, dout], F32,
                                   kind="ExternalInput").ap())
        wr_d.append(nc.dram_tensor(f"wrT{l}", [din, dout], F32,
                                   kind="ExternalInput").ap())
        b_d.append(nc.dram_tensor(f"b{l}", [1, dout], F32,
                                  kind="ExternalInput").ap())
    out_d = nc.dram_tensor("out", [NPP, DIMS[3]], F32, kind="ExternalOutput").ap()

    ystage, yfull = [], []
    for l in range(3):
        dout = DIMS[l + 1]
        ystage.append(nc.dram_tensor(f"ystage{l}", [NPP, dout], BF16,
                                     kind="Internal").ap())
        yfull.append(nc.dram_tensor(f"yfull{l}", [NCORES * NPP, dout], BF16,
                                    kind="Internal", addr_space="Shared").ap())

    with tile.TileContext(nc) as tc:
        with (
            tc.tile_pool(name="const", bufs=1) as cpool,
            tc.tile_pool(name="xpool", bufs=1) as xpool,
            tc.tile_pool(name="wpool", bufs=2) as wpool,
            tc.tile_pool(name="ypool", bufs=3) as ypool,
            tc.tile_pool(name="gpool", bufs=3) as gpool,
            tc.tile_pool(name="ipool", bufs=3) as ipool,
            tc.tile_pool(name="epool", bufs=2) as epool,
            tc.tile_pool(name="psy", bufs=2, space="PSUM") as psy,
            tc.tile_pool(name="pso", bufs=2, space="PSUM") as pso,
            tc.tile_pool(name="pst", bufs=2, space="PSUM") as pst,
        ):
            ident = cpool.tile([P, P], F32)
            make_identity(nc, ident[:])
            ones = cpool.tile([1, P], F32)
            nc.vector.memset(ones[:], 1.0)
            gidx_sb = cpool.tile([P, SW], I16)
            nc.sync.dma_start(out=gidx_sb[:], in_=gidx_d[:])

            xA = xpool.tile([P, 4, NPP], F32, tag="xA")
            xB = xpool.tile([P, 4, NPP], F32, tag="xB")
            nc.sync.dma_start(
                out=xA[:], in_=xT0.rearrange("(t p) m -> p t m", p=P))

            for l in range(3):
                din, dout = DIMS[l], DIMS[l + 1]
                kt = din // P
                xs = [xA, xB, xA][l]
                xd = [xB, xA, None][l]

                wl_sb = wpool.tile([P, kt, dout], F32, tag="wl")
                wr_sb = wpool.tile([P, kt, dout], F32, tag="wr")
                bias_sb = wpool.tile([1, dout], F32, tag="bias")
                nc.sync.dma_start(
                    out=wl_sb[:], in_=wl_d[l].rearrange("(t p) d -> p t d", p=P))
                nc.sync.dma_start(
                    out=wr_sb[:], in_=wr_d[l].rearrange("(t p) d -> p t d", p=P))
                nc.sync.dma_start(out=bias_sb[:], in_=b_d[l][:])

                # ---- Phase A: y = x @ Wl.T for own rows, then AllGather ----
                for m in range(NB):
                    ps_y = psy.tile([P, dout], F32, tag="psy")
                    for k in range(kt):
                        nc.tensor.matmul(
                            ps_y[:],
                            lhsT=xs[:, k, m * P:(m + 1) * P],
                            rhs=wl_sb[:, k, :],
                            start=(k == 0), stop=(k == kt - 1))
                    y_sb = ypool.tile([P, dout], BF16, tag="y")
                    nc.scalar.copy(y_sb[:], ps_y[:])
                    nc.sync.dma_start(
                        out=ystage[l][m * P:(m + 1) * P, :], in_=y_sb[:])

                nc.gpsimd.collective_compute(
                    "AllGather", mybir.AluOpType.bypass,
                    replica_groups=[list(range(NCORES))],
                    ins=[ystage[l][:]], outs=[yfull[l][:]])

                # ---- Phase B: per dst block: gather + segment-mean + Wr + b --
                for b in range(NB):
                    nt = int(K_blk[b]) // P
                    ps_o = pso.tile([P, dout], F32, tag="pso")
                    first = True
                    for c0 in range(0, nt, CT):
                        cs = min(CT, nt - c0)
                        g_sb = gpool.tile([P, CT, dout], BF16, tag="g")
                        nc.gpsimd.dma_gather(
                            out_ap=g_sb[:, :cs, :],
                            in_ap=yfull[l][:],
                            idxs_ap=gidx_sb[:, (int(off[b]) + c0 * P) // 16:
                                            (int(off[b]) + (c0 + cs) * P) // 16],
                            num_idxs=cs * P,
                            num_idxs_reg=cs * P,
                            elem_size=dout)
                        i_sb = ipool.tile([P, CT * P], BF16, tag="i")
                        nc.sync.dma_start(
                            out=i_sb[:, :cs * P],
                            in_=ind_d[:, int(off[b]) + c0 * P:
                                      int(off[b]) + (c0 + cs) * P])
                        for j in range(cs):
                            nc.tensor.matmul(
                                ps_o[:],
                                lhsT=i_sb[:, j * P:(j + 1) * P],
                                rhs=g_sb[:, j, :dout],
                                start=first, stop=False)
                            first = False
                    for k in range(kt):
                        nc.tensor.matmul(
                            ps_o[:],
                            lhsT=xs[:, k, b * P:(b + 1) * P],
                            rhs=wr_sb[:, k, :],
                            start=False, stop=False)
                    nc.tensor.matmul(
                        ps_o[:], lhsT=ones[:1, :], rhs=bias_sb[:1, :],
                        start=False, stop=True)

                    # epilogue: r -> relu(r) / max(||r||, eps)
                    sq = epool.tile([P, DIMS[1]], F32, tag="sq")
                    ssq = epool.tile([P, 1], F32, tag="ssq")
                    nc.scalar.activation(
                        out=sq[:, :dout], in_=ps_o[:],
                        func=mybir.ActivationFunctionType.Square,
                        accum_out=ssq[:])
                    nc.vector.tensor_scalar_max(ssq[:], ssq[:], 1e-24)
                    nrm = epool.tile([P, 1], F32, tag="nrm")
                    nc.scalar.sqrt(nrm[:], ssq[:])
                    rns = epool.tile([P, 1], F32, tag="rns")
                    nc.vector.reciprocal(rns[:], nrm[:])
                    xn = epool.tile([P, DIMS[1]], F32, tag="xn")
                    nc.scalar.activation(
                        out=xn[:, :dout], in_=ps_o[:],
                        func=mybir.ActivationFunctionType.Relu,
                        scale=rns[:])

                    if l < 2:
                        for k in range(dout // P):
                            ps_t = pst.tile([P, P], F32, tag="pst")
                            nc.tensor.transpose(
                                ps_t[:], xn[:, k * P:(k + 1) * P], ident[:])
                            nc.vector.tensor_copy(
                                out=xd[:, k, b * P:(b + 1) * P], in_=ps_t[:])
                    else:
                        nc.sync.dma_start(
                            out=out_d[b * P:(b + 1) * P, :], in_=xn[:, :dout])

    nc.compile()
    return nc


_CACHE = {}


def _prep(x, edge_index, Wl0, b0, Wr0, Wl1, b1, Wr1, Wl2, b2, Wr2):
    """Host preprocessing -> (compiled nc, per-core in_maps)."""
    x = np.asarray(x, dtype=np.float32)
    K_blk, gidx, ind = _preprocess(np.asarray(edge_index))

    key = tuple(K_blk.tolist())
    if key not in _CACHE:
        _CACHE[key] = _build(K_blk)
    nc = _CACHE[key]

    weights = {}
    for l, (Wl, bb, Wr) in enumerate([(Wl0, b0, Wr0), (Wl1, b1, Wr1),
                                      (Wl2, b2, Wr2)]):
        weights[f"wlT{l}"] = np.ascontiguousarray(
            np.asarray(Wl, dtype=np.float32).T)
        weights[f"wrT{l}"] = np.ascontiguousarray(
            np.asarray(Wr, dtype=np.float32).T)
        weights[f"b{l}"] = np.asarray(bb, dtype=np.float32).reshape(1, -1)

    in_maps = []
    for c in range(NCORES):
        xc = np.zeros((NPP, DIMS[0]), dtype=np.float32)
        xc[:NP] = x[c * NP:(c + 1) * NP]
        in_maps.append({
            "xT0": np.ascontiguousarray(xc.T),
            "gidx": np.ascontiguousarray(gidx[c]),
            "ind": np.ascontiguousarray(ind[c]),
            **weights,
        })
    return nc, in_maps


def kernel(x, edge_index, Wl0, b0, Wr0, Wl1, b1, Wr1, Wl2, b2, Wr2):
    nc, in_maps = _prep(x, edge_index, Wl0, b0, Wr0, Wl1, b1, Wr1,
                        Wl2, b2, Wr2)
    res = bass_utils.run_bass_kernel_spmd(
        nc, in_maps, core_ids=list(range(NCORES)))
    out = np.concatenate(
        [res.results[c]["out"][:NP] for c in range(NCORES)], axis=0)
    return out.astype(np.float32)
